# revision 26
# baseline (speedup 1.0000x reference)
"""Trainium2 Bass kernel for nn_BasicRecursiveNN (balanced binary tree RecursiveNN).

Contract: kernel(**inputs) takes the FULL unsharded inputs (numpy arrays, keys as
in reference.setup_inputs()) and returns the full output (outputs, final_state).

Strategy:
  - The balanced tree over 8192 leaves is levelized: with level-major relabeling,
    level t's input X is level t-1's output reshaped (2m,512)->(m,1024), so the
    recursion becomes a chain of dense GEMMs with zero gathers.
  - 8 cores: aligned 1/8 slices per level need no cross-core traffic down to the
    128-node level. The last 127 nodes (levels 64..1) are computed on the
    host during unshard (<0.8% of rows/FLOPs).
  - Per core: indirect-DMA gather of its 1024 leaf embeddings (8 ops pipelined
    with compute); every level runs "flipped" matmuls out = X @ W^T with the
    stationary operand being X^T (so level-0 M-tiles depend only on their own
    two leaf tiles), the bias folded in as a K=1 matmul, tanh on ScalarE into
    node-major fp32 states, and PE transposes + one split-CAST per M-tile
    producing the next level's X^T. Matmuls run in float32r (TF32-like,
    ~1.5e-4 rel err); states/outputs stay fp32.
  - If the index inputs do not match the expected balanced tree, falls back to
    an exact levelized numpy implementation of the reference scan semantics.
"""
import sys
for _p in ("/opt/trn_rl_repo", "/root/.axon_site/_ro/trn_rl_repo"):
    if _p not in sys.path:
        sys.path.insert(0, _p)

import numpy as np

L = 8192
D = 512
NNODES = 2 * L - 1
NCORES = 8
LPC = L // NCORES          # 1024 leaves per core
P = 128

# per-core output-node counts for the 6 device levels (full levels 4096..128)
DEV_LEVELS = [512, 256, 128, 64, 32, 16]
N_INT_DEV = sum(DEV_LEVELS)  # 1008 internal nodes per core on device
HOST_LEVELS = (6, 7, 8, 9, 10, 11, 12)  # full sizes 64..1 -> host


# ---------------------------------------------------------------------------
# host-side tree analysis
# ---------------------------------------------------------------------------

def _build_balanced_tree(num_leaves):
    lefts, rights, post = [], [], []

    def build(lo, hi):
        if hi - lo == 1:
            post.append(lo)
            return lo
        mid = (lo + hi) // 2
        left = build(lo, mid)
        right = build(mid, hi)
        idx = num_leaves + len(lefts)
        lefts.append(left)
        rights.append(right)
        post.append(idx)
        return idx

    sys.setrecursionlimit(100000)
    build(0, num_leaves)
    return (np.asarray(lefts, np.int64), np.asarray(rights, np.int64),
            np.asarray(post, np.int64))


_TREE_CACHE = None


def _tree_info():
    """Expected balanced tree + (level, pos) -> ref-node-id maps."""
    global _TREE_CACHE
    if _TREE_CACHE is not None:
        return _TREE_CACHE
    lefts, rights, post = _build_balanced_tree(L)
    level = np.zeros(NNODES, np.int64)
    lmost = np.zeros(NNODES, np.int64)
    lmost[:L] = np.arange(L)
    for k in range(L - 1):
        lc, rc = lefts[k], rights[k]
        level[L + k] = level[lc] + 1
        lmost[L + k] = lmost[lc]
    # internal node level t (0-based from bottom) and position g
    levels_refids = []
    for t in range(13):
        size = L >> (t + 1)
        levels_refids.append(np.zeros(size, np.int64))
    for k in range(L - 1):
        t = int(level[L + k]) - 1
        g = int(lmost[L + k]) >> (t + 1)
        levels_refids[t][g] = L + k
    _TREE_CACHE = (lefts, rights, post, levels_refids)
    return _TREE_CACHE


# ---------------------------------------------------------------------------
# device program
# ---------------------------------------------------------------------------

_PROGRAM = None
LAST_RESULTS = None


def _build_program():
    import concourse.bass as bass
    import concourse.mybir as mybir
    import concourse.tile as tile
    from contextlib import ExitStack
    from concourse import bacc
    from concourse.masks import make_identity

    F32 = mybir.dt.float32
    F32R = mybir.dt.float32r
    I32 = mybir.dt.int32
    Tanh = mybir.ActivationFunctionType.Tanh

    nc = bacc.Bacc("TRN2", target_bir_lowering=False, debug=False,
                   num_devices=NCORES)

    # tok_perm[p, j]: token for leaf block j (j even: even leaves tile j//2,
    # j odd: odd leaves tile j//2), host-prepared.
    tok_perm = nc.dram_tensor("tok_perm", [P, 8], I32, kind="ExternalInput").ap()
    word_emb = nc.dram_tensor("word_emb", [32000, D], F32, kind="ExternalInput").ap()
    fc_wT = nc.dram_tensor("fc_wT", [2 * D, D], F32R, kind="ExternalInput").ap()
    fc_b = nc.dram_tensor("fc_b", [D], F32, kind="ExternalInput").ap()
    leaf_out = nc.dram_tensor("leaf_out", [LPC, D], F32, kind="ExternalOutput").ap()
    int_out = nc.dram_tensor("int_out", [N_INT_DEV, D], F32R, kind="ExternalOutput").ap()

    with tile.TileContext(nc) as tc, ExitStack() as ctx:
        const = ctx.enter_context(tc.tile_pool(name="const", bufs=1))
        xt_pool = ctx.enter_context(tc.tile_pool(name="xt", bufs=2))
        h_pool = ctx.enter_context(tc.tile_pool(name="hh", bufs=2))
        mm_ps = ctx.enter_context(tc.tile_pool(name="mmps", bufs=3, space="PSUM"))
        tr_ps = ctx.enter_context(tc.tile_pool(name="trps", bufs=4, space="PSUM"))

        # ---- index load + small constants first ----------------------------
        # GpSimd-side constants (memsets, identity) are emitted before the
        # gathers so they occupy the Q7 queue first and the PE warm-up /
        # leaf transposes are never blocked behind the gather stream.
        tokt = const.tile([P, 8], I32, tag="tokt")
        nc.sync.dma_start(out=tokt[:], in_=tok_perm[:, :])
        brow32 = const.tile([1, D], F32, tag="brow32")
        nc.sync.dma_start(out=brow32[:], in_=fc_b[None, :])
        brow = const.tile([1, D], F32R, tag="brow")
        nc.vector.tensor_copy(out=brow[:], in_=brow32[:])
        ones32 = const.tile([1, P], F32, tag="ones32")
        nc.gpsimd.memset(ones32[:], 1.0)
        ones = const.tile([1, P], F32R, tag="ones")
        nc.vector.tensor_copy(out=ones[:], in_=ones32[:])
        ident = const.tile([P, P], F32, tag="ident")
        make_identity(nc, ident[:])
        identr = const.tile([P, P], F32R, tag="identr")
        nc.vector.tensor_copy(out=identr[:], in_=ident[:])

        # PE warm-up: dummy matmuls during the DMA-bound front so the HAM
        # clock gate opens (1.2 -> 2.4 GHz) before real work arrives.
        warm_ps = ctx.enter_context(tc.tile_pool(name="warmps", bufs=1,
                                                 space="PSUM"))
        wps = warm_ps.tile([P, P], F32, tag="warm")
        for _ in range(12):
            nc.tensor.matmul(out=wps[:], lhsT=ident[:], rhs=ident[:],
                             start=True, stop=True)

        # leafAll[p, j, :]: row p of leaf block j. The weight load is queued
        # on the same SWDGE (Q7) stream between gathers 1 and 2, so the first
        # two leaf gathers (which head the critical path) win the SDMA
        # engines, while weight blocks still arrive before the first matmuls.
        leafAll = const.tile([P, 8 * D], F32, tag="leafAll")
        WTall = const.tile([P, 8, D], F32R, tag="WTall")
        for j in range(8):
            nc.gpsimd.indirect_dma_start(
                out=leafAll[:, j * D:(j + 1) * D], out_offset=None, in_=word_emb[:],
                in_offset=bass.IndirectOffsetOnAxis(ap=tokt[:, j:j + 1], axis=0),
            )
            if j == 1:
                nc.gpsimd.dma_start(
                    out=WTall[:], in_=fc_wT.rearrange("(s p) d -> p s d", p=P))

        # ---- levels (all flipped: out H node-major) -------------------------
        # XT layout per level (one tile): global-s block at cols [s*m:(s+1)*m];
        # s<4 = feature block s of left children, s>=4 = of right children.
        # Level-0 XT (m = 512): left children = even leaves (blocks j even);
        # its transposes are emitted inside the level-0 M-tile loop so each
        # M-tile group only waits on its own two leaf gathers.
        XT0 = const.tile([P, 8 * 512], F32R, tag="xt0", name="xt0")
        XT0b = XT0.rearrange("p (b r) -> p b r", r=512)
        XT_prev = XT0
        int_row0 = 0
        for t, m in enumerate(DEV_LEVELS):
            ngrp = max(1, m // P)
            mw = min(m, P)
            H = h_pool.tile([P, ngrp * D], F32R, tag="hh", name=f"h{t}")
            for g in range(ngrp):
                if t == 0:
                    for j in (2 * g, 2 * g + 1):
                        half = j % 2
                        for s in range(4):
                            pst = tr_ps.tile([P, P], F32, tag="trp", space="PSUM",
                                             name=f"pstl{j}_{s}")
                            nc.tensor.transpose(
                                out=pst[:],
                                in_=leafAll[:, j * D + s * P: j * D + (s + 1) * P],
                                identity=ident[:])
                            nc.vector.tensor_copy(
                                out=XT0b[:, s + 4 * half, g * P:(g + 1) * P],
                                in_=pst[:])
                psum = mm_ps.tile([P, D], F32, tag="mm", space="PSUM")
                nc.tensor.matmul(out=psum[:mw, :], lhsT=ones[:1, :mw], rhs=brow[:],
                                 start=True, stop=False)
                for s in range(8):
                    lhsT = XT_prev[:, s * m + g * P: s * m + g * P + mw]
                    nc.tensor.matmul(out=psum[:mw, :], lhsT=lhsT,
                                     rhs=WTall[:, s, :], start=False, stop=(s == 7))
                nc.scalar.activation(out=H[:mw, g * D:(g + 1) * D],
                                     in_=psum[:mw, :], func=Tanh)
            if t == 0:
                # leaf output write (no downstream consumers; overlaps compute)
                nc.sync.dma_start(
                    out=leaf_out.rearrange("(j p) d -> p j d", p=P),
                    in_=leafAll[:].rearrange("p (j d) -> p j d", d=D))
            # write out (natural node order), SWDGE to keep Sync free
            if ngrp == 1:
                nc.gpsimd.dma_start(out=int_out[int_row0:int_row0 + m, :],
                                    in_=H[:m, :D])
            else:
                nc.gpsimd.dma_start(
                    out=int_out[int_row0:int_row0 + m, :].rearrange(
                        "(g p) d -> p g d", p=P),
                    in_=H[:].rearrange("p (g d) -> p g d", d=D))
            int_row0 += m
            # next level's X^T: per-s transpose + split-cast so the next
            # level's s-th matmul only waits on its own block
            if t + 1 < len(DEV_LEVELS):
                mn = m // 2
                XTn = xt_pool.tile([P, 8 * mn], F32R, tag="xtn", name=f"xt{t + 1}")
                XTn4 = XTn.rearrange("p (h s r) -> p h s r", h=2, s=4)
                for g in range(ngrp):
                    for s in range(4):
                        pst = tr_ps.tile([P, P], F32R, tag="trp", space="PSUM",
                                         name=f"pst{t}_{g}_{s}")
                        nc.tensor.transpose(
                            out=pst[:, :mw],
                            in_=H[:mw, g * D + s * P: g * D + (s + 1) * P],
                            identity=identr[:mw, :mw])
                        # pst[p, 2*jj+h] -> XTn[(s+4*h)*mn + g*(mw/2) + jj]
                        nc.vector.tensor_copy(
                            out=XTn4[:, :, s, g * (mw // 2): (g + 1) * (mw // 2)],
                            in_=pst[:, :mw].rearrange("p (jj h) -> p h jj", h=2),
                        )
                XT_prev = XTn
        assert int_row0 == N_INT_DEV

    nc.compile()
    return nc


def _get_program():
    global _PROGRAM
    if _PROGRAM is None:
        _PROGRAM = _build_program()
    return _PROGRAM


# ---------------------------------------------------------------------------
# generic fallback (exact reference scan semantics, levelized numpy)
# ---------------------------------------------------------------------------

def _fallback(tokens, left_idx, right_idx, postorder, word_emb, fc_w, fc_b):
    n_int = left_idx.shape[0]
    n = tokens.shape[0] + n_int
    states = np.zeros((n + 1, word_emb.shape[1]), dtype=np.float32)  # +1 zero row
    states[:tokens.shape[0]] = word_emb[tokens]
    nleaf = tokens.shape[0]
    # levelize with scan visibility: child L+j visible to step i iff j < i
    lvl = np.zeros(n_int, np.int64)
    eff_l = np.empty(n_int, np.int64)
    eff_r = np.empty(n_int, np.int64)
    for i in range(n_int):
        li, ri = int(left_idx[i]), int(right_idx[i])
        vl = li if (li < nleaf or li - nleaf < i) else n   # forward ref -> zeros
        vr = ri if (ri < nleaf or ri - nleaf < i) else n
        eff_l[i] = vl
        eff_r[i] = vr
        dd = 0
        if vl >= nleaf and vl < n:
            dd = max(dd, int(lvl[vl - nleaf]) + 1)
        if vr >= nleaf and vr < n:
            dd = max(dd, int(lvl[vr - nleaf]) + 1)
        lvl[i] = dd
    for t in range(int(lvl.max()) + 1):
        grp = np.nonzero(lvl == t)[0]
        if grp.size == 0:
            continue
        x = np.concatenate([states[eff_l[grp]], states[eff_r[grp]]], axis=1)
        states[nleaf + grp] = np.tanh(x @ fc_w.T + fc_b)
    outputs = states[:n][postorder].astype(np.float32)
    return outputs, outputs[-1].copy()


def _ensure_ntff_hook():
    """Install the axon NTFF profile hook if the image's antenv lacks the
    axon_hooks shim (profiling-only; no effect on execution)."""
    import types
    from concourse import bass_utils
    bass_utils.upload_artifacts = lambda tmpdir: "local://" + str(tmpdir)
    try:
        from antenv import axon_hooks  # noqa: F401
        return
    except ImportError:
        pass
    import antenv
    mod = types.ModuleType("antenv.axon_hooks")
    _state = {"hook": None}
    mod.set_axon_ntff_profile_hook = lambda h: _state.__setitem__("hook", h)
    mod.get_axon_ntff_profile_hook = lambda: _state["hook"]
    sys.modules["antenv.axon_hooks"] = mod
    antenv.axon_hooks = mod
    try:
        from trn_agent_boot.trn_boot import _ntff_profile_via_ctypes
        hook = _ntff_profile_via_ctypes("/opt/axon/libaxon_pjrt.so")
        if hook is not None:
            mod.set_axon_ntff_profile_hook(hook)
    except Exception:
        pass


# ---------------------------------------------------------------------------
# entry point
# ---------------------------------------------------------------------------

def kernel(tokens, left_idx, right_idx, postorder, word_emb, fc_w, fc_b):
    tokens = np.asarray(tokens)
    left_idx = np.asarray(left_idx, np.int64)
    right_idx = np.asarray(right_idx, np.int64)
    postorder = np.asarray(postorder, np.int64)
    word_emb = np.ascontiguousarray(np.asarray(word_emb, np.float32))
    fc_w = np.asarray(fc_w, np.float32)
    fc_b = np.asarray(fc_b, np.float32)

    lefts, rights, post, levels_refids = _tree_info()
    structured = (
        tokens.shape == (L,) and word_emb.shape == (32000, D)
        and fc_w.shape == (D, 2 * D)
        and np.array_equal(left_idx, lefts)
        and np.array_equal(rights, right_idx)
        and np.array_equal(post, postorder)
    )
    if not structured:
        return _fallback(tokens, left_idx, right_idx, postorder, word_emb,
                         fc_w, fc_b)

    import os
    from concourse import bass_utils

    if os.environ.get("KERNEL_TRACE") == "1":
        _ensure_ntff_hook()
    nc = _get_program()
    fc_wT = np.ascontiguousarray(fc_w.T)
    tok32 = tokens.astype(np.int32)
    in_maps = []
    for c in range(NCORES):
        sl = tok32[c * LPC:(c + 1) * LPC]
        # tok_perm[p, j]: j even -> even leaves tile j//2, j odd -> odd tile
        tp = np.empty((P, 8), np.int32)
        for j in range(8):
            g, half = j // 2, j % 2
            tp[:, j] = sl[2 * (g * P + np.arange(P)) + half]
        in_maps.append({
            "tok_perm": tp,
            "word_emb": word_emb,
            "fc_wT": fc_wT,
            "fc_b": fc_b,
        })
    kwargs = {}
    if os.environ.get("KERNEL_TRACE") == "1":
        kwargs["trace"] = True
    res = bass_utils.run_bass_kernel_spmd(nc, in_maps,
                                          core_ids=list(range(NCORES)), **kwargs)
    global LAST_RESULTS
    LAST_RESULTS = res

    # ---- unshard / assemble ------------------------------------------------
    states = np.zeros((NNODES, D), np.float32)
    for c in range(NCORES):
        lo = res.results[c]["leaf_out"]
        io = res.results[c]["int_out"]
        base = c * LPC
        # leaf row j*128+p  <->  leaf base + 2*(j//2*128 + p) + (j % 2)
        for j in range(8):
            g, half = j // 2, j % 2
            states[base + 2 * (g * P + np.arange(P)) + half] = \
                lo[j * P:(j + 1) * P]
        off = 0
        for t, m in enumerate(DEV_LEVELS):
            g = c * m + np.arange(m)
            states[levels_refids[t][g]] = io[off:off + m]
            off += m
    # host tail: full levels 16, 8, 4, 2, 1
    for t in HOST_LEVELS:
        for refid in levels_refids[t]:
            lc, rc = lefts[refid - L], rights[refid - L]
            x = np.concatenate([states[lc], states[rc]])
            states[refid] = np.tanh(fc_w @ x + fc_b)
    outputs = states[postorder]
    final_state = outputs[-1].copy()
    return outputs, final_state


# revision 27
# speedup vs baseline: 1.0014x; 1.0014x over previous
"""Trainium2 Bass kernel for nn_BasicRecursiveNN (balanced binary tree RecursiveNN).

Contract: kernel(**inputs) takes the FULL unsharded inputs (numpy arrays, keys as
in reference.setup_inputs()) and returns the full output (outputs, final_state).

Strategy:
  - The balanced tree over 8192 leaves is levelized: with level-major relabeling,
    level t's input X is level t-1's output reshaped (2m,512)->(m,1024), so the
    recursion becomes a chain of dense GEMMs with zero gathers.
  - 8 cores: aligned 1/8 slices per level need no cross-core traffic down to the
    128-node level. The last 127 nodes (levels 64..1) are computed on the
    host during unshard (<0.8% of rows/FLOPs).
  - Per core: indirect-DMA gather of its 1024 leaf embeddings (8 ops pipelined
    with compute); every level runs "flipped" matmuls out = X @ W^T with the
    stationary operand being X^T (so level-0 M-tiles depend only on their own
    two leaf tiles), the bias folded in as a K=1 matmul, tanh on ScalarE into
    node-major fp32 states, and PE transposes + one split-CAST per M-tile
    producing the next level's X^T. Matmuls run in float32r (TF32-like,
    ~1.5e-4 rel err); states/outputs stay fp32.
  - If the index inputs do not match the expected balanced tree, falls back to
    an exact levelized numpy implementation of the reference scan semantics.
"""
import sys
for _p in ("/opt/trn_rl_repo", "/root/.axon_site/_ro/trn_rl_repo"):
    if _p not in sys.path:
        sys.path.insert(0, _p)

import numpy as np

L = 8192
D = 512
NNODES = 2 * L - 1
NCORES = 8
LPC = L // NCORES          # 1024 leaves per core
P = 128

# per-core output-node counts for the 6 device levels (full levels 4096..128)
DEV_LEVELS = [512, 256, 128, 64, 32, 16]
N_INT_DEV = sum(DEV_LEVELS)  # 1008 internal nodes per core on device
HOST_LEVELS = (6, 7, 8, 9, 10, 11, 12)  # full sizes 64..1 -> host


# ---------------------------------------------------------------------------
# host-side tree analysis
# ---------------------------------------------------------------------------

def _build_balanced_tree(num_leaves):
    lefts, rights, post = [], [], []

    def build(lo, hi):
        if hi - lo == 1:
            post.append(lo)
            return lo
        mid = (lo + hi) // 2
        left = build(lo, mid)
        right = build(mid, hi)
        idx = num_leaves + len(lefts)
        lefts.append(left)
        rights.append(right)
        post.append(idx)
        return idx

    sys.setrecursionlimit(100000)
    build(0, num_leaves)
    return (np.asarray(lefts, np.int64), np.asarray(rights, np.int64),
            np.asarray(post, np.int64))


_TREE_CACHE = None


def _tree_info():
    """Expected balanced tree + (level, pos) -> ref-node-id maps."""
    global _TREE_CACHE
    if _TREE_CACHE is not None:
        return _TREE_CACHE
    lefts, rights, post = _build_balanced_tree(L)
    level = np.zeros(NNODES, np.int64)
    lmost = np.zeros(NNODES, np.int64)
    lmost[:L] = np.arange(L)
    for k in range(L - 1):
        lc, rc = lefts[k], rights[k]
        level[L + k] = level[lc] + 1
        lmost[L + k] = lmost[lc]
    # internal node level t (0-based from bottom) and position g
    levels_refids = []
    for t in range(13):
        size = L >> (t + 1)
        levels_refids.append(np.zeros(size, np.int64))
    for k in range(L - 1):
        t = int(level[L + k]) - 1
        g = int(lmost[L + k]) >> (t + 1)
        levels_refids[t][g] = L + k
    _TREE_CACHE = (lefts, rights, post, levels_refids)
    return _TREE_CACHE


# ---------------------------------------------------------------------------
# device program
# ---------------------------------------------------------------------------

_PROGRAM = {}
LAST_RESULTS = None


def _build_program(with_bias):
    import concourse.bass as bass
    import concourse.mybir as mybir
    import concourse.tile as tile
    from contextlib import ExitStack
    from concourse import bacc
    from concourse.masks import make_identity

    F32 = mybir.dt.float32
    F32R = mybir.dt.float32r
    I32 = mybir.dt.int32
    Tanh = mybir.ActivationFunctionType.Tanh

    nc = bacc.Bacc("TRN2", target_bir_lowering=False, debug=False,
                   num_devices=NCORES)

    # tok_perm[p, j]: token for leaf block j (j even: even leaves tile j//2,
    # j odd: odd leaves tile j//2), host-prepared.
    tok_perm = nc.dram_tensor("tok_perm", [P, 8], I32, kind="ExternalInput").ap()
    word_emb = nc.dram_tensor("word_emb", [32000, D], F32, kind="ExternalInput").ap()
    fc_wT = nc.dram_tensor("fc_wT", [2 * D, D], F32R, kind="ExternalInput").ap()
    fc_b = nc.dram_tensor("fc_b", [D], F32, kind="ExternalInput").ap()
    leaf_out = nc.dram_tensor("leaf_out", [LPC, D], F32, kind="ExternalOutput").ap()
    int_out = nc.dram_tensor("int_out", [N_INT_DEV, D], F32R, kind="ExternalOutput").ap()

    with tile.TileContext(nc) as tc, ExitStack() as ctx:
        const = ctx.enter_context(tc.tile_pool(name="const", bufs=1))
        xt_pool = ctx.enter_context(tc.tile_pool(name="xt", bufs=2))
        h_pool = ctx.enter_context(tc.tile_pool(name="hh", bufs=2))
        mm_ps = ctx.enter_context(tc.tile_pool(name="mmps", bufs=3, space="PSUM"))
        tr_ps = ctx.enter_context(tc.tile_pool(name="trps", bufs=4, space="PSUM"))

        # ---- index load + small constants first ----------------------------
        # GpSimd-side constants (memsets, identity) are emitted before the
        # gathers so they occupy the Q7 queue first and the PE warm-up /
        # leaf transposes are never blocked behind the gather stream.
        tokt = const.tile([P, 8], I32, tag="tokt")
        nc.sync.dma_start(out=tokt[:], in_=tok_perm[:, :])
        if with_bias:
            brow32 = const.tile([1, D], F32, tag="brow32")
            nc.sync.dma_start(out=brow32[:], in_=fc_b[None, :])
            brow = const.tile([1, D], F32R, tag="brow")
            nc.vector.tensor_copy(out=brow[:], in_=brow32[:])
            ones32 = const.tile([1, P], F32, tag="ones32")
            nc.gpsimd.memset(ones32[:], 1.0)
            ones = const.tile([1, P], F32R, tag="ones")
            nc.vector.tensor_copy(out=ones[:], in_=ones32[:])
        ident = const.tile([P, P], F32, tag="ident")
        make_identity(nc, ident[:])
        identr = const.tile([P, P], F32R, tag="identr")
        nc.vector.tensor_copy(out=identr[:], in_=ident[:])

        # PE warm-up: dummy matmuls during the DMA-bound front so the HAM
        # clock gate opens (1.2 -> 2.4 GHz) before real work arrives.
        warm_ps = ctx.enter_context(tc.tile_pool(name="warmps", bufs=1,
                                                 space="PSUM"))
        wps = warm_ps.tile([P, P], F32, tag="warm")
        for _ in range(12):
            nc.tensor.matmul(out=wps[:], lhsT=ident[:], rhs=ident[:],
                             start=True, stop=True)

        # leafAll[p, j, :]: row p of leaf block j. The weight load is queued
        # on the same SWDGE (Q7) stream between gathers 1 and 2, so the first
        # two leaf gathers (which head the critical path) win the SDMA
        # engines, while weight blocks still arrive before the first matmuls.
        leafAll = const.tile([P, 8 * D], F32, tag="leafAll")
        WTall = const.tile([P, 8, D], F32R, tag="WTall")
        for j in range(8):
            nc.gpsimd.indirect_dma_start(
                out=leafAll[:, j * D:(j + 1) * D], out_offset=None, in_=word_emb[:],
                in_offset=bass.IndirectOffsetOnAxis(ap=tokt[:, j:j + 1], axis=0),
            )
            if j == 1:
                nc.gpsimd.dma_start(
                    out=WTall[:], in_=fc_wT.rearrange("(s p) d -> p s d", p=P))

        # ---- levels (all flipped: out H node-major) -------------------------
        # XT layout per level (one tile): global-s block at cols [s*m:(s+1)*m];
        # s<4 = feature block s of left children, s>=4 = of right children.
        # Level-0 XT (m = 512): left children = even leaves (blocks j even);
        # its transposes are emitted inside the level-0 M-tile loop so each
        # M-tile group only waits on its own two leaf gathers.
        XT0 = const.tile([P, 8 * 512], F32R, tag="xt0", name="xt0")
        XT0b = XT0.rearrange("p (b r) -> p b r", r=512)
        XT_prev = XT0
        int_row0 = 0
        for t, m in enumerate(DEV_LEVELS):
            ngrp = max(1, m // P)
            mw = min(m, P)
            H = h_pool.tile([P, ngrp * D], F32R, tag="hh", name=f"h{t}")
            for g in range(ngrp):
                if t == 0:
                    for j in (2 * g, 2 * g + 1):
                        half = j % 2
                        for s in range(4):
                            pst = tr_ps.tile([P, P], F32, tag="trp", space="PSUM",
                                             name=f"pstl{j}_{s}")
                            nc.tensor.transpose(
                                out=pst[:],
                                in_=leafAll[:, j * D + s * P: j * D + (s + 1) * P],
                                identity=ident[:])
                            nc.vector.tensor_copy(
                                out=XT0b[:, s + 4 * half, g * P:(g + 1) * P],
                                in_=pst[:])
                psum = mm_ps.tile([P, D], F32, tag="mm", space="PSUM")
                if with_bias:
                    # fold the bias in as a K=1 matmul (clears psum too)
                    nc.tensor.matmul(out=psum[:mw, :], lhsT=ones[:1, :mw],
                                     rhs=brow[:], start=True, stop=False)
                for s in range(8):
                    lhsT = XT_prev[:, s * m + g * P: s * m + g * P + mw]
                    nc.tensor.matmul(out=psum[:mw, :], lhsT=lhsT,
                                     rhs=WTall[:, s, :],
                                     start=(s == 0 and not with_bias),
                                     stop=(s == 7))
                nc.scalar.activation(out=H[:mw, g * D:(g + 1) * D],
                                     in_=psum[:mw, :], func=Tanh)
            if t == 0:
                # leaf output write (no downstream consumers; overlaps compute)
                nc.sync.dma_start(
                    out=leaf_out.rearrange("(j p) d -> p j d", p=P),
                    in_=leafAll[:].rearrange("p (j d) -> p j d", d=D))
            # write out (natural node order), SWDGE to keep Sync free
            if ngrp == 1:
                nc.gpsimd.dma_start(out=int_out[int_row0:int_row0 + m, :],
                                    in_=H[:m, :D])
            else:
                nc.gpsimd.dma_start(
                    out=int_out[int_row0:int_row0 + m, :].rearrange(
                        "(g p) d -> p g d", p=P),
                    in_=H[:].rearrange("p (g d) -> p g d", d=D))
            int_row0 += m
            # next level's X^T: per-s transpose + split-cast so the next
            # level's s-th matmul only waits on its own block
            if t + 1 < len(DEV_LEVELS):
                mn = m // 2
                XTn = xt_pool.tile([P, 8 * mn], F32R, tag="xtn", name=f"xt{t + 1}")
                XTn4 = XTn.rearrange("p (h s r) -> p h s r", h=2, s=4)
                for g in range(ngrp):
                    for s in range(4):
                        pst = tr_ps.tile([P, P], F32R, tag="trp", space="PSUM",
                                         name=f"pst{t}_{g}_{s}")
                        nc.tensor.transpose(
                            out=pst[:, :mw],
                            in_=H[:mw, g * D + s * P: g * D + (s + 1) * P],
                            identity=identr[:mw, :mw])
                        # pst[p, 2*jj+h] -> XTn[(s+4*h)*mn + g*(mw/2) + jj]
                        nc.vector.tensor_copy(
                            out=XTn4[:, :, s, g * (mw // 2): (g + 1) * (mw // 2)],
                            in_=pst[:, :mw].rearrange("p (jj h) -> p h jj", h=2),
                        )
                XT_prev = XTn
        assert int_row0 == N_INT_DEV

    nc.compile()
    return nc


def _get_program(with_bias):
    if with_bias not in _PROGRAM:
        _PROGRAM[with_bias] = _build_program(with_bias)
    return _PROGRAM[with_bias]


# ---------------------------------------------------------------------------
# generic fallback (exact reference scan semantics, levelized numpy)
# ---------------------------------------------------------------------------

def _fallback(tokens, left_idx, right_idx, postorder, word_emb, fc_w, fc_b):
    n_int = left_idx.shape[0]
    n = tokens.shape[0] + n_int
    states = np.zeros((n + 1, word_emb.shape[1]), dtype=np.float32)  # +1 zero row
    states[:tokens.shape[0]] = word_emb[tokens]
    nleaf = tokens.shape[0]
    # levelize with scan visibility: child L+j visible to step i iff j < i
    lvl = np.zeros(n_int, np.int64)
    eff_l = np.empty(n_int, np.int64)
    eff_r = np.empty(n_int, np.int64)
    for i in range(n_int):
        li, ri = int(left_idx[i]), int(right_idx[i])
        vl = li if (li < nleaf or li - nleaf < i) else n   # forward ref -> zeros
        vr = ri if (ri < nleaf or ri - nleaf < i) else n
        eff_l[i] = vl
        eff_r[i] = vr
        dd = 0
        if vl >= nleaf and vl < n:
            dd = max(dd, int(lvl[vl - nleaf]) + 1)
        if vr >= nleaf and vr < n:
            dd = max(dd, int(lvl[vr - nleaf]) + 1)
        lvl[i] = dd
    for t in range(int(lvl.max()) + 1):
        grp = np.nonzero(lvl == t)[0]
        if grp.size == 0:
            continue
        x = np.concatenate([states[eff_l[grp]], states[eff_r[grp]]], axis=1)
        states[nleaf + grp] = np.tanh(x @ fc_w.T + fc_b)
    outputs = states[:n][postorder].astype(np.float32)
    return outputs, outputs[-1].copy()


def _ensure_ntff_hook():
    """Install the axon NTFF profile hook if the image's antenv lacks the
    axon_hooks shim (profiling-only; no effect on execution)."""
    import types
    from concourse import bass_utils
    bass_utils.upload_artifacts = lambda tmpdir: "local://" + str(tmpdir)
    try:
        from antenv import axon_hooks  # noqa: F401
        return
    except ImportError:
        pass
    import antenv
    mod = types.ModuleType("antenv.axon_hooks")
    _state = {"hook": None}
    mod.set_axon_ntff_profile_hook = lambda h: _state.__setitem__("hook", h)
    mod.get_axon_ntff_profile_hook = lambda: _state["hook"]
    sys.modules["antenv.axon_hooks"] = mod
    antenv.axon_hooks = mod
    try:
        from trn_agent_boot.trn_boot import _ntff_profile_via_ctypes
        hook = _ntff_profile_via_ctypes("/opt/axon/libaxon_pjrt.so")
        if hook is not None:
            mod.set_axon_ntff_profile_hook(hook)
    except Exception:
        pass


# ---------------------------------------------------------------------------
# entry point
# ---------------------------------------------------------------------------

def kernel(tokens, left_idx, right_idx, postorder, word_emb, fc_w, fc_b):
    tokens = np.asarray(tokens)
    left_idx = np.asarray(left_idx, np.int64)
    right_idx = np.asarray(right_idx, np.int64)
    postorder = np.asarray(postorder, np.int64)
    word_emb = np.ascontiguousarray(np.asarray(word_emb, np.float32))
    fc_w = np.asarray(fc_w, np.float32)
    fc_b = np.asarray(fc_b, np.float32)

    lefts, rights, post, levels_refids = _tree_info()
    structured = (
        tokens.shape == (L,) and word_emb.shape == (32000, D)
        and fc_w.shape == (D, 2 * D)
        and np.array_equal(left_idx, lefts)
        and np.array_equal(rights, right_idx)
        and np.array_equal(post, postorder)
    )
    if not structured:
        return _fallback(tokens, left_idx, right_idx, postorder, word_emb,
                         fc_w, fc_b)

    import os
    from concourse import bass_utils

    if os.environ.get("KERNEL_TRACE") == "1":
        _ensure_ntff_hook()
    # the reference problem has fc_b == 0; elide the per-M-tile bias matmul
    # in that case (the general biased program remains available)
    nc = _get_program(bool(np.any(fc_b != 0)))
    fc_wT = np.ascontiguousarray(fc_w.T)
    tok32 = tokens.astype(np.int32)
    in_maps = []
    for c in range(NCORES):
        sl = tok32[c * LPC:(c + 1) * LPC]
        # tok_perm[p, j]: j even -> even leaves tile j//2, j odd -> odd tile
        tp = np.empty((P, 8), np.int32)
        for j in range(8):
            g, half = j // 2, j % 2
            tp[:, j] = sl[2 * (g * P + np.arange(P)) + half]
        in_maps.append({
            "tok_perm": tp,
            "word_emb": word_emb,
            "fc_wT": fc_wT,
            "fc_b": fc_b,
        })
    kwargs = {}
    if os.environ.get("KERNEL_TRACE") == "1":
        kwargs["trace"] = True
    res = bass_utils.run_bass_kernel_spmd(nc, in_maps,
                                          core_ids=list(range(NCORES)), **kwargs)
    global LAST_RESULTS
    LAST_RESULTS = res

    # ---- unshard / assemble ------------------------------------------------
    states = np.zeros((NNODES, D), np.float32)
    for c in range(NCORES):
        lo = res.results[c]["leaf_out"]
        io = res.results[c]["int_out"]
        base = c * LPC
        # leaf row j*128+p  <->  leaf base + 2*(j//2*128 + p) + (j % 2)
        for j in range(8):
            g, half = j // 2, j % 2
            states[base + 2 * (g * P + np.arange(P)) + half] = \
                lo[j * P:(j + 1) * P]
        off = 0
        for t, m in enumerate(DEV_LEVELS):
            g = c * m + np.arange(m)
            states[levels_refids[t][g]] = io[off:off + m]
            off += m
    # host tail: full levels 16, 8, 4, 2, 1
    for t in HOST_LEVELS:
        for refid in levels_refids[t]:
            lc, rc = lefts[refid - L], rights[refid - L]
            x = np.concatenate([states[lc], states[rc]])
            states[refid] = np.tanh(fc_w @ x + fc_b)
    outputs = states[postorder]
    final_state = outputs[-1].copy()
    return outputs, final_state


# revision 28
# speedup vs baseline: 1.0327x; 1.0313x over previous
"""Trainium2 Bass kernel for nn_BasicRecursiveNN (balanced binary tree RecursiveNN).

Contract: kernel(**inputs) takes the FULL unsharded inputs (numpy arrays, keys as
in reference.setup_inputs()) and returns the full output (outputs, final_state).

Strategy:
  - The balanced tree over 8192 leaves is levelized: with level-major relabeling,
    level t's input X is level t-1's output reshaped (2m,512)->(m,1024), so the
    recursion becomes a chain of dense GEMMs with zero gathers.
  - 8 cores: aligned 1/8 slices per level need no cross-core traffic down to the
    128-node level. The last 127 nodes (levels 64..1) are computed on the
    host during unshard (<0.8% of rows/FLOPs).
  - Per core: indirect-DMA gather of its 1024 leaf embeddings (8 ops pipelined
    with compute); every level runs "flipped" matmuls out = X @ W^T with the
    stationary operand being X^T (so level-0 M-tiles depend only on their own
    two leaf tiles), the bias folded in as a K=1 matmul, tanh on ScalarE into
    node-major fp32 states, and PE transposes + one split-CAST per M-tile
    producing the next level's X^T. Matmuls run in float32r (TF32-like,
    ~1.5e-4 rel err); states/outputs stay fp32.
  - If the index inputs do not match the expected balanced tree, falls back to
    an exact levelized numpy implementation of the reference scan semantics.
"""
import sys
for _p in ("/opt/trn_rl_repo", "/root/.axon_site/_ro/trn_rl_repo"):
    if _p not in sys.path:
        sys.path.insert(0, _p)

import numpy as np

L = 8192
D = 512
NNODES = 2 * L - 1
NCORES = 8
LPC = L // NCORES          # 1024 leaves per core
P = 128

# per-core output-node counts for the 6 device levels (full levels 4096..128)
DEV_LEVELS = [512, 256, 128, 64, 32, 16]
N_INT_DEV = sum(DEV_LEVELS)  # 1008 internal nodes per core on device
HOST_LEVELS = (6, 7, 8, 9, 10, 11, 12)  # full sizes 64..1 -> host


# ---------------------------------------------------------------------------
# host-side tree analysis
# ---------------------------------------------------------------------------

def _build_balanced_tree(num_leaves):
    lefts, rights, post = [], [], []

    def build(lo, hi):
        if hi - lo == 1:
            post.append(lo)
            return lo
        mid = (lo + hi) // 2
        left = build(lo, mid)
        right = build(mid, hi)
        idx = num_leaves + len(lefts)
        lefts.append(left)
        rights.append(right)
        post.append(idx)
        return idx

    sys.setrecursionlimit(100000)
    build(0, num_leaves)
    return (np.asarray(lefts, np.int64), np.asarray(rights, np.int64),
            np.asarray(post, np.int64))


_TREE_CACHE = None


def _tree_info():
    """Expected balanced tree + (level, pos) -> ref-node-id maps."""
    global _TREE_CACHE
    if _TREE_CACHE is not None:
        return _TREE_CACHE
    lefts, rights, post = _build_balanced_tree(L)
    level = np.zeros(NNODES, np.int64)
    lmost = np.zeros(NNODES, np.int64)
    lmost[:L] = np.arange(L)
    for k in range(L - 1):
        lc, rc = lefts[k], rights[k]
        level[L + k] = level[lc] + 1
        lmost[L + k] = lmost[lc]
    # internal node level t (0-based from bottom) and position g
    levels_refids = []
    for t in range(13):
        size = L >> (t + 1)
        levels_refids.append(np.zeros(size, np.int64))
    for k in range(L - 1):
        t = int(level[L + k]) - 1
        g = int(lmost[L + k]) >> (t + 1)
        levels_refids[t][g] = L + k
    _TREE_CACHE = (lefts, rights, post, levels_refids)
    return _TREE_CACHE


# ---------------------------------------------------------------------------
# device program
# ---------------------------------------------------------------------------

_PROGRAM = {}
LAST_RESULTS = None


def _build_program(with_bias):
    import concourse.bass as bass
    import concourse.mybir as mybir
    import concourse.tile as tile
    from contextlib import ExitStack
    from concourse import bacc
    from concourse.masks import make_identity

    F32 = mybir.dt.float32
    F32R = mybir.dt.float32r
    I32 = mybir.dt.int32
    Tanh = mybir.ActivationFunctionType.Tanh

    nc = bacc.Bacc("TRN2", target_bir_lowering=False, debug=False,
                   num_devices=NCORES)

    # tok_perm[p, j]: token for leaf block j (j even: even leaves tile j//2,
    # j odd: odd leaves tile j//2), host-prepared.
    tok_perm = nc.dram_tensor("tok_perm", [P, 8], I32, kind="ExternalInput").ap()
    word_emb = nc.dram_tensor("word_emb", [32000, D], F32, kind="ExternalInput").ap()
    fc_wT = nc.dram_tensor("fc_wT", [2 * D, D], F32R, kind="ExternalInput").ap()
    fc_b = nc.dram_tensor("fc_b", [D], F32, kind="ExternalInput").ap()
    leaf_out = nc.dram_tensor("leaf_out", [LPC, D], F32, kind="ExternalOutput").ap()
    int_out = nc.dram_tensor("int_out", [N_INT_DEV, D], F32R, kind="ExternalOutput").ap()

    with tile.TileContext(nc) as tc, ExitStack() as ctx:
        const = ctx.enter_context(tc.tile_pool(name="const", bufs=1))
        xt_pool = ctx.enter_context(tc.tile_pool(name="xt", bufs=2))
        h_pool = ctx.enter_context(tc.tile_pool(name="hh", bufs=2))
        mm_ps = ctx.enter_context(tc.tile_pool(name="mmps", bufs=3, space="PSUM"))
        tr_ps = ctx.enter_context(tc.tile_pool(name="trps", bufs=4, space="PSUM"))

        # ---- index load + small constants first ----------------------------
        # GpSimd-side constants (memsets, identity) are emitted before the
        # gathers so they occupy the Q7 queue first and the PE warm-up /
        # leaf transposes are never blocked behind the gather stream.
        tokt = const.tile([P, 8], I32, tag="tokt")
        nc.sync.dma_start(out=tokt[:], in_=tok_perm[:, :])
        if with_bias:
            brow32 = const.tile([1, D], F32, tag="brow32")
            nc.sync.dma_start(out=brow32[:], in_=fc_b[None, :])
            brow = const.tile([1, D], F32R, tag="brow")
            nc.vector.tensor_copy(out=brow[:], in_=brow32[:])
            ones32 = const.tile([1, P], F32, tag="ones32")
            nc.gpsimd.memset(ones32[:], 1.0)
            ones = const.tile([1, P], F32R, tag="ones")
            nc.vector.tensor_copy(out=ones[:], in_=ones32[:])
        ident = const.tile([P, P], F32, tag="ident")
        make_identity(nc, ident[:])
        identr = const.tile([P, P], F32R, tag="identr")
        nc.vector.tensor_copy(out=identr[:], in_=ident[:])

        # PE warm-up: dummy matmuls during the DMA-bound front so the HAM
        # clock gate opens (1.2 -> 2.4 GHz) before real work arrives.
        warm_ps = ctx.enter_context(tc.tile_pool(name="warmps", bufs=1,
                                                 space="PSUM"))
        wps = warm_ps.tile([P, P], F32, tag="warm")
        for _ in range(12):
            nc.tensor.matmul(out=wps[:], lhsT=ident[:], rhs=ident[:],
                             start=True, stop=True)

        # leafAll[p, j, :]: row p of leaf block j. The weight load is queued
        # on the same SWDGE (Q7) stream between gathers 1 and 2, so the first
        # two leaf gathers (which head the critical path) win the SDMA
        # engines, while weight blocks still arrive before the first matmuls.
        leafAll = const.tile([P, 8 * D], F32, tag="leafAll")
        WTall = const.tile([P, 8, D], F32R, tag="WTall")
        for j in range(8):
            nc.gpsimd.indirect_dma_start(
                out=leafAll[:, j * D:(j + 1) * D], out_offset=None, in_=word_emb[:],
                in_offset=bass.IndirectOffsetOnAxis(ap=tokt[:, j:j + 1], axis=0),
            )
            if j == 1:
                nc.gpsimd.dma_start(
                    out=WTall[:], in_=fc_wT.rearrange("(s p) d -> p s d", p=P))

        # ---- levels (all flipped: out H node-major) -------------------------
        # XT layout per level (one tile): global-s block at cols [s*m:(s+1)*m];
        # s<4 = feature block s of left children, s>=4 = of right children.
        # Level-0 XT (m = 512): left children = even leaves (blocks j even);
        # its transposes are emitted inside the level-0 M-tile loop so each
        # M-tile group only waits on its own two leaf gathers.
        XT0 = const.tile([P, 8 * 512], F32R, tag="xt0", name="xt0")
        XT0b = XT0.rearrange("p (b r) -> p b r", r=512)
        XT_prev = XT0
        int_row0 = 0
        for t, m in enumerate(DEV_LEVELS):
            ngrp = max(1, m // P)
            mw = min(m, P)
            H = h_pool.tile([P, ngrp * D], F32R, tag="hh", name=f"h{t}")
            for g in range(ngrp):
                if t == 0:
                    for j in (2 * g, 2 * g + 1):
                        half = j % 2
                        for s in range(4):
                            pst = tr_ps.tile([P, P], F32, tag="trp", space="PSUM",
                                             name=f"pstl{j}_{s}")
                            nc.tensor.transpose(
                                out=pst[:],
                                in_=leafAll[:, j * D + s * P: j * D + (s + 1) * P],
                                identity=ident[:])
                            nc.vector.tensor_copy(
                                out=XT0b[:, s + 4 * half, g * P:(g + 1) * P],
                                in_=pst[:])
                psum = mm_ps.tile([P, D], F32, tag="mm", space="PSUM")
                if with_bias:
                    # fold the bias in as a K=1 matmul (clears psum too)
                    nc.tensor.matmul(out=psum[:mw, :], lhsT=ones[:1, :mw],
                                     rhs=brow[:], start=True, stop=False)
                for s in range(8):
                    lhsT = XT_prev[:, s * m + g * P: s * m + g * P + mw]
                    nc.tensor.matmul(out=psum[:mw, :], lhsT=lhsT,
                                     rhs=WTall[:, s, :],
                                     start=(s == 0 and not with_bias),
                                     stop=(s == 7))
                nc.scalar.activation(out=H[:mw, g * D:(g + 1) * D],
                                     in_=psum[:mw, :], func=Tanh)
                if t == 0 and g < 2:
                    # dependency-free HAM bridge: keep the PE busy signal alive
                    # while the next M-tile waits on its leaf gathers
                    for _ in range(3):
                        nc.tensor.matmul(out=wps[:], lhsT=ident[:],
                                         rhs=ident[:], start=True, stop=True)
            if t == 0:
                # leaf output write (no downstream consumers; overlaps compute)
                nc.sync.dma_start(
                    out=leaf_out.rearrange("(j p) d -> p j d", p=P),
                    in_=leafAll[:].rearrange("p (j d) -> p j d", d=D))
            # write out (natural node order), SWDGE to keep Sync free
            if ngrp == 1:
                nc.gpsimd.dma_start(out=int_out[int_row0:int_row0 + m, :],
                                    in_=H[:m, :D])
            else:
                nc.gpsimd.dma_start(
                    out=int_out[int_row0:int_row0 + m, :].rearrange(
                        "(g p) d -> p g d", p=P),
                    in_=H[:].rearrange("p (g d) -> p g d", d=D))
            int_row0 += m
            # next level's X^T: per-s transpose + split-cast so the next
            # level's s-th matmul only waits on its own block
            if t + 1 < len(DEV_LEVELS):
                mn = m // 2
                XTn = xt_pool.tile([P, 8 * mn], F32R, tag="xtn", name=f"xt{t + 1}")
                XTn4 = XTn.rearrange("p (h s r) -> p h s r", h=2, s=4)
                for g in range(ngrp):
                    for s in range(4):
                        pst = tr_ps.tile([P, P], F32R, tag="trp", space="PSUM",
                                         name=f"pst{t}_{g}_{s}")
                        nc.tensor.transpose(
                            out=pst[:, :mw],
                            in_=H[:mw, g * D + s * P: g * D + (s + 1) * P],
                            identity=identr[:mw, :mw])
                        # pst[p, 2*jj+h] -> XTn[(s+4*h)*mn + g*(mw/2) + jj]
                        nc.vector.tensor_copy(
                            out=XTn4[:, :, s, g * (mw // 2): (g + 1) * (mw // 2)],
                            in_=pst[:, :mw].rearrange("p (jj h) -> p h jj", h=2),
                        )
                XT_prev = XTn
        assert int_row0 == N_INT_DEV

    nc.compile()
    return nc


def _get_program(with_bias):
    if with_bias not in _PROGRAM:
        _PROGRAM[with_bias] = _build_program(with_bias)
    return _PROGRAM[with_bias]


# ---------------------------------------------------------------------------
# generic fallback (exact reference scan semantics, levelized numpy)
# ---------------------------------------------------------------------------

def _fallback(tokens, left_idx, right_idx, postorder, word_emb, fc_w, fc_b):
    n_int = left_idx.shape[0]
    n = tokens.shape[0] + n_int
    states = np.zeros((n + 1, word_emb.shape[1]), dtype=np.float32)  # +1 zero row
    states[:tokens.shape[0]] = word_emb[tokens]
    nleaf = tokens.shape[0]
    # levelize with scan visibility: child L+j visible to step i iff j < i
    lvl = np.zeros(n_int, np.int64)
    eff_l = np.empty(n_int, np.int64)
    eff_r = np.empty(n_int, np.int64)
    for i in range(n_int):
        li, ri = int(left_idx[i]), int(right_idx[i])
        vl = li if (li < nleaf or li - nleaf < i) else n   # forward ref -> zeros
        vr = ri if (ri < nleaf or ri - nleaf < i) else n
        eff_l[i] = vl
        eff_r[i] = vr
        dd = 0
        if vl >= nleaf and vl < n:
            dd = max(dd, int(lvl[vl - nleaf]) + 1)
        if vr >= nleaf and vr < n:
            dd = max(dd, int(lvl[vr - nleaf]) + 1)
        lvl[i] = dd
    for t in range(int(lvl.max()) + 1):
        grp = np.nonzero(lvl == t)[0]
        if grp.size == 0:
            continue
        x = np.concatenate([states[eff_l[grp]], states[eff_r[grp]]], axis=1)
        states[nleaf + grp] = np.tanh(x @ fc_w.T + fc_b)
    outputs = states[:n][postorder].astype(np.float32)
    return outputs, outputs[-1].copy()


def _ensure_ntff_hook():
    """Install the axon NTFF profile hook if the image's antenv lacks the
    axon_hooks shim (profiling-only; no effect on execution)."""
    import types
    from concourse import bass_utils
    bass_utils.upload_artifacts = lambda tmpdir: "local://" + str(tmpdir)
    try:
        from antenv import axon_hooks  # noqa: F401
        return
    except ImportError:
        pass
    import antenv
    mod = types.ModuleType("antenv.axon_hooks")
    _state = {"hook": None}
    mod.set_axon_ntff_profile_hook = lambda h: _state.__setitem__("hook", h)
    mod.get_axon_ntff_profile_hook = lambda: _state["hook"]
    sys.modules["antenv.axon_hooks"] = mod
    antenv.axon_hooks = mod
    try:
        from trn_agent_boot.trn_boot import _ntff_profile_via_ctypes
        hook = _ntff_profile_via_ctypes("/opt/axon/libaxon_pjrt.so")
        if hook is not None:
            mod.set_axon_ntff_profile_hook(hook)
    except Exception:
        pass


# ---------------------------------------------------------------------------
# entry point
# ---------------------------------------------------------------------------

def kernel(tokens, left_idx, right_idx, postorder, word_emb, fc_w, fc_b):
    tokens = np.asarray(tokens)
    left_idx = np.asarray(left_idx, np.int64)
    right_idx = np.asarray(right_idx, np.int64)
    postorder = np.asarray(postorder, np.int64)
    word_emb = np.ascontiguousarray(np.asarray(word_emb, np.float32))
    fc_w = np.asarray(fc_w, np.float32)
    fc_b = np.asarray(fc_b, np.float32)

    lefts, rights, post, levels_refids = _tree_info()
    structured = (
        tokens.shape == (L,) and word_emb.shape == (32000, D)
        and fc_w.shape == (D, 2 * D)
        and np.array_equal(left_idx, lefts)
        and np.array_equal(rights, right_idx)
        and np.array_equal(post, postorder)
    )
    if not structured:
        return _fallback(tokens, left_idx, right_idx, postorder, word_emb,
                         fc_w, fc_b)

    import os
    from concourse import bass_utils

    if os.environ.get("KERNEL_TRACE") == "1":
        _ensure_ntff_hook()
    # the reference problem has fc_b == 0; elide the per-M-tile bias matmul
    # in that case (the general biased program remains available)
    nc = _get_program(bool(np.any(fc_b != 0)))
    fc_wT = np.ascontiguousarray(fc_w.T)
    tok32 = tokens.astype(np.int32)
    in_maps = []
    for c in range(NCORES):
        sl = tok32[c * LPC:(c + 1) * LPC]
        # tok_perm[p, j]: j even -> even leaves tile j//2, j odd -> odd tile
        tp = np.empty((P, 8), np.int32)
        for j in range(8):
            g, half = j // 2, j % 2
            tp[:, j] = sl[2 * (g * P + np.arange(P)) + half]
        in_maps.append({
            "tok_perm": tp,
            "word_emb": word_emb,
            "fc_wT": fc_wT,
            "fc_b": fc_b,
        })
    kwargs = {}
    if os.environ.get("KERNEL_TRACE") == "1":
        kwargs["trace"] = True
    res = bass_utils.run_bass_kernel_spmd(nc, in_maps,
                                          core_ids=list(range(NCORES)), **kwargs)
    global LAST_RESULTS
    LAST_RESULTS = res

    # ---- unshard / assemble ------------------------------------------------
    states = np.zeros((NNODES, D), np.float32)
    for c in range(NCORES):
        lo = res.results[c]["leaf_out"]
        io = res.results[c]["int_out"]
        base = c * LPC
        # leaf row j*128+p  <->  leaf base + 2*(j//2*128 + p) + (j % 2)
        for j in range(8):
            g, half = j // 2, j % 2
            states[base + 2 * (g * P + np.arange(P)) + half] = \
                lo[j * P:(j + 1) * P]
        off = 0
        for t, m in enumerate(DEV_LEVELS):
            g = c * m + np.arange(m)
            states[levels_refids[t][g]] = io[off:off + m]
            off += m
    # host tail: full levels 16, 8, 4, 2, 1
    for t in HOST_LEVELS:
        for refid in levels_refids[t]:
            lc, rc = lefts[refid - L], rights[refid - L]
            x = np.concatenate([states[lc], states[rc]])
            states[refid] = np.tanh(fc_w @ x + fc_b)
    outputs = states[postorder]
    final_state = outputs[-1].copy()
    return outputs, final_state
